# revision 12
# baseline (speedup 1.0000x reference)
"""Trilinear interpolation (grid_sample) on 8 TRN2 NeuronCores.

The axon tunnel moves ~40MB/s, so wall time is dominated by bytes shipped.
Strategy (vs. shipping a host-built 1GB expanded f32 table):
- Host: quantize the (16,128,128,128) volume to int8 (sym, scale amax/127),
  transpose to channel-last, slice into 8 x-slabs of 17 planes (4.45MB/core).
- Host: bin the 1M points by x-window (2 planes) -> 64 bins, 8 per core;
  pad each bin to a chunk multiple; ship coords once in planeA layout as
  i16 fixed-point (x window-local *2^14; y/z *2^9 biased by -2^15).
- Device: DMA-expand the int8 slab into an 8-corner row table in device DRAM
  (row = 8 corners x 16 ch int8 = 128B payload in a 256B-stride row), with
  y/z edge-clamp handled by split DMAs. DVE computes floor/frac/weights and
  int16 row indices from planeA coords; a DRAM round-trip permutes indices
  into the gpsimd 16-partition-wrapped 8-replica layout; one 256B dma_gather
  per point; DVE converts payload i8->f32, applies corner weights,
  tree-reduces, clamps to [-127,127] and emits int8 output (the HW
  f32->i8 convert rounds to nearest).
- Host: inverse-permute, dequantize by the volume scale.
"""
import numpy as np

import concourse.bass as bass
import concourse.tile as tile
from concourse import bacc, mybir
from concourse import bass_utils

P = 128
C = 16              # channels
D = 128             # grid size per dim
CH = 8192           # points per gather chunk
ROWB = 256          # bytes per table row (128B payload + 128B pad)
PAY = 128           # payload bytes per row (8 corners * 16 ch int8)
WINDOW = 2 * D * D  # rows per gather window (2 x-planes) = 32768
NCORES = 8
XPL = D // NCORES   # x-planes per core = 16
PLANE = D * D       # 16384 rows per x-plane

_cache = {}
RUN_CORES = 8   # override <8 for debugging: only first k cores run on HW
LAST_EXEC_S = 0.0


def _build(nch, cpb):
    """Build the SPMD Bass program. nch = chunks per core, cpb = chunks per
    bin."""
    U = nch * (CH // P)        # planeA cols per partition (nch*64)
    S = CH // P                # slots per partition per chunk = 64
    TBL = nch * (CH // 16)     # idx table cols (i16), nch*512
    f32 = mybir.dt.float32
    i32 = mybir.dt.int32
    i16 = mybir.dt.int16
    i8 = mybir.dt.int8
    gt = mybir.AluOpType.is_gt

    nc = bacc.Bacc("TRN2", target_bir_lowering=False, debug=False,
                   num_devices=RUN_CORES)
    slab = nc.dram_tensor("slab", [17 * PLANE, C], i8, kind="ExternalInput")
    # fixed-point coords, one i16 plane per dim: x window-local *16384,
    # y/z *512 biased by -32768
    pco = nc.dram_tensor("pco", [P, 3 * U], i16, kind="ExternalInput")
    out = nc.dram_tensor("out", [P, U * C], i8, kind="ExternalOutput")

    def view(ap, dims):
        return bass.AP(ap.tensor, ap.offset, [ap.ap[0]] + dims)

    with tile.TileContext(nc) as tc:
        with tc.tile_pool(name="persist", bufs=1) as pp, \
             tc.tile_pool(name="dram", bufs=1, space="DRAM") as dp:
            # 67MB corner-expanded row table in device DRAM
            table = dp.tile([P, XPL * PLANE * ROWB // P], i8)
            tidx = pp.tile([P, TBL], i16)
            w8 = pp.tile([P, U * 8], f32)
            tv = table[:]
            sv = slab.ap()

            # ---------- corner expansion: slab -> table (DRAM->DRAM) ------
            # table[(x,y,z)*256 + j*16 + ch] = slab[(x+dx, min(y+dy,127),
            #   min(z+dz,127)), ch] for corner j = dx*4+dy*2+dz
            # (DMA APs max 3 dims -> one DMA per x-plane per piece)
            for dx in range(2):
                for dy in range(2):
                    for dz in range(2):
                        j = dx * 4 + dy * 2 + dz
                        ypieces = ([(0, D - 1, 1), (D - 1, D, 0)] if dy
                                   else [(0, D, 0)])
                        zpieces = ([(0, D - 1, 1), (D - 1, D, 0)] if dz
                                   else [(0, D, 0)])
                        for (ys, ye, yd) in ypieces:
                            for (zs, ze, zd) in zpieces:
                                for xl in range(XPL):
                                    woff = (xl * PLANE + ys * D + zs) \
                                        * ROWB + j * C
                                    wdims = [[D * ROWB, ye - ys],
                                             [ROWB, ze - zs], [1, C]]
                                    roff = ((xl + dx) * PLANE
                                            + (ys + yd) * D + (zs + zd)) * C
                                    rdims = [[D * C, ye - ys],
                                             [C, ze - zs], [1, C]]
                                    nc.sync.dma_start(
                                        bass.AP(tv.tensor, tv.offset + woff,
                                                wdims),
                                        bass.AP(sv.tensor, sv.offset + roff,
                                                rdims))

            # ---------- prep: floors/fracs/weights/indices (planeA) -------
            with tc.tile_pool(name="prep", bufs=1) as pa:
                pc = pa.tile([P, 3 * U], i16)
                nc.sync.dma_start(pc[:], pco.ap())

                def floor_frac(d, bias, scale, name):
                    cc = pa.tile([P, U], f32, tag=f"c{name}")
                    nc.vector.tensor_copy(cc[:], pc[:, d * U:(d + 1) * U])
                    nc.vector.tensor_scalar(cc[:], cc[:], bias, scale,
                                            mybir.AluOpType.add,
                                            mybir.AluOpType.mult)
                    fi = pa.tile([P, U], i32, tag=f"fi{name}")
                    nc.vector.tensor_copy(fi[:], cc[:])
                    ff = pa.tile([P, U], f32, tag=f"ff{name}")
                    nc.vector.tensor_copy(ff[:], fi[:])
                    adj = pa.tile([P, U], f32, tag=f"adj{name}")
                    nc.vector.tensor_tensor(adj[:], ff[:], cc[:], gt)
                    nc.vector.tensor_sub(ff[:], ff[:], adj[:])
                    nc.vector.tensor_sub(cc[:], cc[:], ff[:])   # cc = frac
                    return cc, ff

                frx, ffx = floor_frac(0, 0.0, 1.0 / 16384.0, "x")
                fry, ffy = floor_frac(1, 32768.0, 1.0 / 512.0, "y")
                frz, ffz = floor_frac(2, 32768.0, 1.0 / 512.0, "z")

                # idx = px*16384 + ffy*128 + ffz  (fits int16: <= 32767;
                # px = ffx is already the window-local x in {0,1})
                nc.vector.tensor_scalar_mul(ffx[:], ffx[:], float(PLANE))
                nc.vector.tensor_scalar(ffy[:], ffy[:], float(D), None,
                                        mybir.AluOpType.mult)
                nc.vector.tensor_add(ffx[:], ffx[:], ffy[:])
                nc.vector.tensor_add(ffx[:], ffx[:], ffz[:])
                idxi = pa.tile([P, U], i32)
                nc.vector.tensor_copy(idxi[:], ffx[:])
                idx16 = pa.tile([P, U], i16)
                nc.vector.tensor_copy(idx16[:], idxi[:])

                # DRAM round-trip permute into the gpsimd idx-table layout:
                # elem m of chunk k lives at (part 16g + m%16, col k*512 +
                # m//16) for all 8 replicas g. planeA slot (p, k*64+a) is
                # elem m = a*128+p -> L[(p%16)*TBL + k*512 + 8a + p//16]
                L = dp.tile([16, TBL], i16)
                lv = L[:]
                for b in range(8):
                    nc.sync.dma_start(
                        bass.AP(lv.tensor, lv.offset + b,
                                [[TBL, 16], [CH // 16, nch], [8, S]]),
                        idx16[:][16 * b:16 * (b + 1), :])
                for g in range(8):
                    dst = tidx[:][16 * g:16 * (g + 1), :]
                    nc.sync.dma_start(
                        dst, bass.AP(lv.tensor, lv.offset,
                                     [[TBL, 16], [1, TBL]]))

                # corner weights w8[j] = wx[dx]*wy[dy]*wz[dz], j=dx*4+dy*2+dz
                def wpair(fr, name):
                    w = pa.tile([P, U * 2], f32, tag=f"w{name}")
                    wv = w[:].rearrange("p (u two) -> p u two", two=2)
                    nc.vector.tensor_scalar(wv[:, :, 0], fr[:], -1.0, 1.0,
                                            mybir.AluOpType.mult,
                                            mybir.AluOpType.add)
                    nc.vector.tensor_copy(wv[:, :, 1], fr[:])
                    return w

                WX, WY, WZ = wpair(frx, "x"), wpair(fry, "y"), wpair(frz, "z")
                wyz = pa.tile([P, U * 4], f32)
                ay = WY[:]; az = WZ[:]
                nc.vector.tensor_mul(
                    bass.AP(wyz[:].tensor, wyz[:].offset,
                            [wyz[:].ap[0], [4, U], [2, 2], [1, 2]]),
                    bass.AP(ay.tensor, ay.offset,
                            [ay.ap[0], [2, U], [1, 2], [0, 2]]),
                    bass.AP(az.tensor, az.offset,
                            [az.ap[0], [2, U], [0, 2], [1, 2]]))
                ax = WX[:]; ayz = wyz[:]
                nc.vector.tensor_mul(
                    bass.AP(w8[:].tensor, w8[:].offset,
                            [w8[:].ap[0], [8, U], [4, 2], [1, 4]]),
                    bass.AP(ax.tensor, ax.offset,
                            [ax.ap[0], [2, U], [1, 2], [0, 4]]),
                    bass.AP(ayz.tensor, ayz.offset,
                            [ayz.ap[0], [4, U], [0, 2], [1, 4]]))

            # ---------- main loop: gather + weight + reduce + emit --------
            with tc.tile_pool(name="g", bufs=2) as gp, \
                 tc.tile_pool(name="f", bufs=1) as fp, \
                 tc.tile_pool(name="red", bufs=1) as rp, \
                 tc.tile_pool(name="o", bufs=2) as op_:
                for k in range(nch):
                    w = k // cpb
                    g = gp.tile([P, S * ROWB], i8, tag="g")
                    g3 = g[:].rearrange("p (s e) -> p s e", e=ROWB)
                    win = bass.AP(tv.tensor, tv.offset + w * WINDOW * ROWB,
                                  [[ROWB, WINDOW], [1, ROWB]])
                    nc.gpsimd.dma_gather(
                        out_ap=g3, in_ap=win,
                        idxs_ap=tidx[:, k * (CH // 16):(k + 1) * (CH // 16)],
                        num_idxs=CH, num_idxs_reg=CH, elem_size=ROWB,
                        single_packet=False)
                    gf = fp.tile([P, S * PAY], f32, tag="gf")
                    nc.vector.tensor_copy(
                        view(gf[:], [[PAY, S], [1, PAY]]),
                        view(g[:], [[ROWB, S], [1, PAY]]))
                    gv4 = view(gf[:], [[PAY, S], [C, 8], [1, C]])
                    w8v = view(w8[:, k * S * 8:(k + 1) * S * 8],
                               [[8, S], [1, 8], [0, C]])
                    nc.vector.tensor_mul(gv4, gv4, w8v)
                    s1 = rp.tile([P, S * 64], f32, tag="s1")
                    nc.vector.tensor_add(
                        view(s1[:], [[64, S], [1, 64]]),
                        view(gf[:], [[PAY, S], [1, 64]]),
                        view(gf[:, 64:], [[PAY, S], [1, 64]]))
                    s2 = rp.tile([P, S * 32], f32, tag="s2")
                    nc.vector.tensor_add(
                        view(s2[:], [[32, S], [1, 32]]),
                        view(s1[:], [[64, S], [1, 32]]),
                        view(s1[:, 32:], [[64, S], [1, 32]]))
                    ot = rp.tile([P, S * C], f32, tag="ot")
                    nc.vector.tensor_add(
                        view(ot[:], [[C, S], [1, C]]),
                        view(s2[:], [[32, S], [1, C]]),
                        view(s2[:, C:], [[32, S], [1, C]]))
                    # clamp to [-127,127]; the HW f32->i8 convert rounds to
                    # nearest (verified empirically; CoreSim truncates)
                    nc.vector.tensor_scalar_max(ot[:], ot[:], -127.0)
                    nc.vector.tensor_scalar_min(ot[:], ot[:], 127.0)
                    oti = op_.tile([P, S * C], i8, tag="oti")
                    nc.vector.tensor_copy(oti[:], ot[:])
                    nc.sync.dma_start(
                        out.ap()[:, k * S * C:(k + 1) * S * C], oti[:])
    nc.compile()
    return nc


def kernel(input, coords):
    input = np.asarray(input, dtype=np.float32)
    coords = np.asarray(coords, dtype=np.float32)
    N = coords.shape[0]

    # ---- int8 quantization (symmetric) + channel-last layout ----
    amax = float(np.abs(input).max())
    if amax == 0.0:
        amax = 1.0
    s = np.float32(amax / 127.0)
    qi = np.rint(input * np.float32(127.0 / amax)).astype(np.int8)
    Qv = np.ascontiguousarray(qi.reshape(C, -1).T).reshape(D, PLANE, C)

    # ---- binning by x-window (exact same f32 math as the device) ----
    cx = (coords[:, 0] + np.float32(1.0)) * np.float32(63.5)
    fx = np.floor(cx).astype(np.int64)
    np.clip(fx, 0, D - 2, out=fx)
    key = fx >> 1                        # global window 0..63
    order = np.argsort(key, kind="stable")
    counts = np.bincount(key, minlength=64)
    capb = max(CH, int(np.ceil(counts.max() / CH)) * CH)
    cpb = capb // CH
    nch = 8 * cpb
    U = nch * (CH // P)

    i_all = np.full(64 * capb, -1, np.int64)  # padded slot -> orig idx
    starts = np.zeros(65, np.int64)
    np.cumsum(counts, out=starts[1:])
    for gb in range(64):
        n = int(counts[gb])
        i_all[gb * capb:gb * capb + n] = order[starts[gb]:starts[gb] + n]

    in_maps = []
    core_meta = []
    binidx = np.arange(8 * capb) // capb
    for c in range(NCORES):
        ids = i_all[c * 8 * capb:(c + 1) * 8 * capb]
        valid = ids >= 0
        # pad coords: center of the bin's first plane, y=z=center
        cc = np.empty((ids.size, 3), np.float32)
        cc[:, 0] = (2 * (8 * c + binidx) + 0.5) / np.float32(63.5) - 1.0
        cc[:, 1:] = 0.0
        cc[valid] = coords[ids[valid]]
        # i16 fixed-point: x window-local *2^14, y/z *2^9 biased -2^15
        ccd = (cc + np.float32(1.0)) * np.float32(63.5)
        xb = (2.0 * (8 * c + binidx)).astype(np.float32)
        q = np.empty((ids.size, 3), np.int16)
        q[:, 0] = np.clip(np.rint((ccd[:, 0] - xb) * np.float32(16384.0)),
                          0, 32767).astype(np.int16)
        q[:, 1] = (np.clip(np.rint(ccd[:, 1] * np.float32(512.0)),
                           0, 65023) - 32768).astype(np.int16)
        q[:, 2] = (np.clip(np.rint(ccd[:, 2] * np.float32(512.0)),
                           0, 65023) - 32768).astype(np.int16)
        # planeA: slot i = k*CH + a*128 + p -> (p, k*64+a)
        arr = q.reshape(nch, CH // P, P, 3).transpose(3, 2, 0, 1)
        pco = np.empty((P, 3 * U), np.int16)
        for d in range(3):
            pco[:, d * U:(d + 1) * U] = arr[d].reshape(P, U)
        if c < NCORES - 1:
            slab = Qv[XPL * c:XPL * c + 17]     # zero-copy view
        else:
            slab = Qv[np.clip(np.arange(XPL * c, XPL * c + 17), 0, D - 1)]
        in_maps.append({
            "slab": slab.reshape(17 * PLANE, C),
            "pco": pco,
        })
        core_meta.append((ids, valid))

    key_cfg = (nch, cpb)
    if key_cfg not in _cache:
        _cache.clear()
        _cache[key_cfg] = _build(nch, cpb)
    nc = _cache[key_cfg]

    import time as _time
    _t0 = _time.perf_counter()
    res = bass_utils.run_bass_kernel_spmd(
        nc, in_maps[:RUN_CORES], core_ids=list(range(RUN_CORES)))
    global LAST_EXEC_S
    LAST_EXEC_S = _time.perf_counter() - _t0
    if RUN_CORES < NCORES:
        z = np.zeros_like(res.results[0]["out"])
        res.results = list(res.results) + [
            {"out": z} for _ in range(NCORES - RUN_CORES)]

    outf = np.empty((C, N), np.float32)
    for c in range(NCORES):
        ids, valid = core_meta[c]
        vals = res.results[c]["out"].reshape(P, nch, CH // P, C)
        vals = vals.transpose(1, 2, 0, 3).reshape(-1, C)
        outf[:, ids[valid]] = (vals[valid].astype(np.float32) * s).T
    return outf


# revision 16
# speedup vs baseline: 1.0131x; 1.0131x over previous
"""Trilinear interpolation (grid_sample) on 8 TRN2 NeuronCores.

The axon tunnel moves ~40MB/s, so wall time is dominated by bytes shipped.
Strategy (vs. shipping a host-built 1GB expanded f32 table):
- Host: quantize the (16,128,128,128) volume to int8 (sym, scale amax/127),
  transpose to channel-last, slice into 8 x-slabs of 17 planes (4.45MB/core).
- Host: bin the 1M points by x-window (2 planes) -> 64 bins, 8 per core;
  pad each bin to a chunk multiple; ship coords once in planeA layout as
  i16 fixed-point (x window-local *2^14; y/z *2^9 biased by -2^15).
- Device: DMA-expand the int8 slab into an 8-corner row table in device DRAM
  (row = 8 corners x 16 ch int8 = 128B payload in a 256B-stride row), with
  y/z edge-clamp handled by split DMAs. DVE computes floor/frac/weights and
  int16 row indices from planeA coords; a DRAM round-trip permutes indices
  into the gpsimd 16-partition-wrapped 8-replica layout; one 256B dma_gather
  per point; DVE converts payload i8->f32, applies corner weights,
  tree-reduces, clamps to [-127,127] and emits int8 output (the HW
  f32->i8 convert rounds to nearest).
- Host: inverse-permute, dequantize by the volume scale.
"""
import numpy as np

import concourse.bass as bass
import concourse.tile as tile
from concourse import bacc, mybir
from concourse import bass_utils

P = 128
C = 16              # channels
D = 128             # grid size per dim
CH = 8192           # points per gather chunk
ROWB = 256          # bytes per table row (128B payload + 128B pad)
PAY = 128           # payload bytes per row (8 corners * 16 ch int8)
WINDOW = 2 * D * D  # rows per gather window (2 x-planes) = 32768
NCORES = 8
XPL = D // NCORES   # x-planes per core = 16
PLANE = D * D       # 16384 rows per x-plane

_cache = {}
RUN_CORES = 8   # override <8 for debugging: only first k cores run on HW
LAST_EXEC_S = 0.0


def _build(nch, cpb):
    """Build the SPMD Bass program. nch = chunks per core, cpb = chunks per
    bin."""
    U = nch * (CH // P)        # planeA cols per partition (nch*64)
    S = CH // P                # slots per partition per chunk = 64
    TBL = nch * (CH // 16)     # idx table cols (i16), nch*512
    f32 = mybir.dt.float32
    i32 = mybir.dt.int32
    i16 = mybir.dt.int16
    i8 = mybir.dt.int8
    gt = mybir.AluOpType.is_gt

    nc = bacc.Bacc("TRN2", target_bir_lowering=False, debug=False,
                   num_devices=RUN_CORES)
    slab = nc.dram_tensor("slab", [17 * PLANE, C], i8, kind="ExternalInput")
    # fixed-point coords, one i16 plane per dim: x window-local *16384,
    # y/z *512 biased by -32768
    pco = nc.dram_tensor("pco", [P, 3 * U], i16, kind="ExternalInput")
    out = nc.dram_tensor("out", [P, U * C], i8, kind="ExternalOutput")

    def view(ap, dims):
        return bass.AP(ap.tensor, ap.offset, [ap.ap[0]] + dims)

    with tile.TileContext(nc) as tc:
        with tc.tile_pool(name="persist", bufs=1) as pp, \
             tc.tile_pool(name="dram", bufs=1, space="DRAM") as dp:
            # 67MB corner-expanded row table in device DRAM
            table = dp.tile([P, XPL * PLANE * ROWB // P], i8)
            tidx = pp.tile([P, TBL], i16)
            w8 = pp.tile([P, U * 8], f32)
            tv = table[:]
            sv = slab.ap()

            # ---------- corner expansion: slab -> table (DRAM->DRAM) ------
            # table[(x,y,z)*256 + j*16 + ch] = slab[(x+dx, min(y+dy,127),
            #   min(z+dz,127)), ch] for corner j = dx*4+dy*2+dz
            # (DMA APs max 3 dims -> one DMA per x-plane per piece)
            for dx in range(2):
                for dy in range(2):
                    for dz in range(2):
                        j = dx * 4 + dy * 2 + dz
                        ypieces = ([(0, D - 1, 1), (D - 1, D, 0)] if dy
                                   else [(0, D, 0)])
                        zpieces = ([(0, D - 1, 1), (D - 1, D, 0)] if dz
                                   else [(0, D, 0)])
                        for (ys, ye, yd) in ypieces:
                            for (zs, ze, zd) in zpieces:
                                for xl in range(XPL):
                                    woff = (xl * PLANE + ys * D + zs) \
                                        * ROWB + j * C
                                    wdims = [[D * ROWB, ye - ys],
                                             [ROWB, ze - zs], [1, C]]
                                    roff = ((xl + dx) * PLANE
                                            + (ys + yd) * D + (zs + zd)) * C
                                    rdims = [[D * C, ye - ys],
                                             [C, ze - zs], [1, C]]
                                    nc.sync.dma_start(
                                        bass.AP(tv.tensor, tv.offset + woff,
                                                wdims),
                                        bass.AP(sv.tensor, sv.offset + roff,
                                                rdims))

            # ---------- prep: floors/fracs/weights/indices (planeA) -------
            with tc.tile_pool(name="prep", bufs=1) as pa:
                pc = pa.tile([P, 3 * U], i16)
                nc.sync.dma_start(pc[:], pco.ap())

                def floor_frac(d, bias, scale, name):
                    cc = pa.tile([P, U], f32, tag=f"c{name}")
                    nc.vector.tensor_copy(cc[:], pc[:, d * U:(d + 1) * U])
                    nc.vector.tensor_scalar(cc[:], cc[:], bias, scale,
                                            mybir.AluOpType.add,
                                            mybir.AluOpType.mult)
                    fi = pa.tile([P, U], i32, tag=f"fi{name}")
                    nc.vector.tensor_copy(fi[:], cc[:])
                    ff = pa.tile([P, U], f32, tag=f"ff{name}")
                    nc.vector.tensor_copy(ff[:], fi[:])
                    adj = pa.tile([P, U], f32, tag=f"adj{name}")
                    nc.vector.tensor_tensor(adj[:], ff[:], cc[:], gt)
                    nc.vector.tensor_sub(ff[:], ff[:], adj[:])
                    nc.vector.tensor_sub(cc[:], cc[:], ff[:])   # cc = frac
                    return cc, ff

                frx, ffx = floor_frac(0, 0.0, 1.0 / 16384.0, "x")
                fry, ffy = floor_frac(1, 32768.0, 1.0 / 512.0, "y")
                frz, ffz = floor_frac(2, 32768.0, 1.0 / 512.0, "z")

                # idx = px*16384 + ffy*128 + ffz  (fits int16: <= 32767;
                # px = ffx is already the window-local x in {0,1})
                nc.vector.tensor_scalar_mul(ffx[:], ffx[:], float(PLANE))
                nc.vector.tensor_scalar(ffy[:], ffy[:], float(D), None,
                                        mybir.AluOpType.mult)
                nc.vector.tensor_add(ffx[:], ffx[:], ffy[:])
                nc.vector.tensor_add(ffx[:], ffx[:], ffz[:])
                idxi = pa.tile([P, U], i32)
                nc.vector.tensor_copy(idxi[:], ffx[:])
                idx16 = pa.tile([P, U], i16)
                nc.vector.tensor_copy(idx16[:], idxi[:])

                # DRAM round-trip permute into the gpsimd idx-table layout:
                # elem m of chunk k lives at (part 16g + m%16, col k*512 +
                # m//16) for all 8 replicas g. planeA slot (p, k*64+a) is
                # elem m = a*128+p -> L[(p%16)*TBL + k*512 + 8a + p//16]
                L = dp.tile([16, TBL], i16)
                lv = L[:]
                for b in range(8):
                    nc.sync.dma_start(
                        bass.AP(lv.tensor, lv.offset + b,
                                [[TBL, 16], [CH // 16, nch], [8, S]]),
                        idx16[:][16 * b:16 * (b + 1), :])
                for g in range(8):
                    dst = tidx[:][16 * g:16 * (g + 1), :]
                    nc.sync.dma_start(
                        dst, bass.AP(lv.tensor, lv.offset,
                                     [[TBL, 16], [1, TBL]]))

                # corner weights w8[j] = wx[dx]*wy[dy]*wz[dz], j=dx*4+dy*2+dz
                def wpair(fr, name):
                    w = pa.tile([P, U * 2], f32, tag=f"w{name}")
                    wv = w[:].rearrange("p (u two) -> p u two", two=2)
                    nc.vector.tensor_scalar(wv[:, :, 0], fr[:], -1.0, 1.0,
                                            mybir.AluOpType.mult,
                                            mybir.AluOpType.add)
                    nc.vector.tensor_copy(wv[:, :, 1], fr[:])
                    return w

                WX, WY, WZ = wpair(frx, "x"), wpair(fry, "y"), wpair(frz, "z")
                wyz = pa.tile([P, U * 4], f32)
                ay = WY[:]; az = WZ[:]
                nc.vector.tensor_mul(
                    bass.AP(wyz[:].tensor, wyz[:].offset,
                            [wyz[:].ap[0], [4, U], [2, 2], [1, 2]]),
                    bass.AP(ay.tensor, ay.offset,
                            [ay.ap[0], [2, U], [1, 2], [0, 2]]),
                    bass.AP(az.tensor, az.offset,
                            [az.ap[0], [2, U], [0, 2], [1, 2]]))
                ax = WX[:]; ayz = wyz[:]
                nc.vector.tensor_mul(
                    bass.AP(w8[:].tensor, w8[:].offset,
                            [w8[:].ap[0], [8, U], [4, 2], [1, 4]]),
                    bass.AP(ax.tensor, ax.offset,
                            [ax.ap[0], [2, U], [1, 2], [0, 4]]),
                    bass.AP(ayz.tensor, ayz.offset,
                            [ayz.ap[0], [4, U], [0, 2], [1, 4]]))

            # ---------- main loop: gather + weight + reduce + emit --------
            with tc.tile_pool(name="g", bufs=2) as gp, \
                 tc.tile_pool(name="f", bufs=1) as fp, \
                 tc.tile_pool(name="red", bufs=1) as rp, \
                 tc.tile_pool(name="o", bufs=2) as op_:
                for k in range(nch):
                    w = k // cpb
                    g = gp.tile([P, S * ROWB], i8, tag="g")
                    g3 = g[:].rearrange("p (s e) -> p s e", e=ROWB)
                    win = bass.AP(tv.tensor, tv.offset + w * WINDOW * ROWB,
                                  [[ROWB, WINDOW], [1, ROWB]])
                    nc.gpsimd.dma_gather(
                        out_ap=g3, in_ap=win,
                        idxs_ap=tidx[:, k * (CH // 16):(k + 1) * (CH // 16)],
                        num_idxs=CH, num_idxs_reg=CH, elem_size=ROWB,
                        single_packet=False)
                    gf = fp.tile([P, S * PAY], f32, tag="gf")
                    nc.vector.tensor_copy(
                        view(gf[:], [[PAY, S], [1, PAY]]),
                        view(g[:], [[ROWB, S], [1, PAY]]))
                    gv4 = view(gf[:], [[PAY, S], [C, 8], [1, C]])
                    w8v = view(w8[:, k * S * 8:(k + 1) * S * 8],
                               [[8, S], [1, 8], [0, C]])
                    nc.vector.tensor_mul(gv4, gv4, w8v)
                    s1 = rp.tile([P, S * 64], f32, tag="s1")
                    nc.vector.tensor_add(
                        view(s1[:], [[64, S], [1, 64]]),
                        view(gf[:], [[PAY, S], [1, 64]]),
                        view(gf[:, 64:], [[PAY, S], [1, 64]]))
                    s2 = rp.tile([P, S * 32], f32, tag="s2")
                    nc.vector.tensor_add(
                        view(s2[:], [[32, S], [1, 32]]),
                        view(s1[:], [[64, S], [1, 32]]),
                        view(s1[:, 32:], [[64, S], [1, 32]]))
                    ot = rp.tile([P, S * C], f32, tag="ot")
                    nc.vector.tensor_add(
                        view(ot[:], [[C, S], [1, C]]),
                        view(s2[:], [[32, S], [1, C]]),
                        view(s2[:, C:], [[32, S], [1, C]]))
                    # clamp to [-127,127]; the HW f32->i8 convert rounds to
                    # nearest (verified empirically; CoreSim truncates)
                    nc.vector.tensor_scalar_max(ot[:], ot[:], -127.0)
                    nc.vector.tensor_scalar_min(ot[:], ot[:], 127.0)
                    oti = op_.tile([P, S * C], i8, tag="oti")
                    nc.vector.tensor_copy(oti[:], ot[:])
                    nc.sync.dma_start(
                        out.ap()[:, k * S * C:(k + 1) * S * C], oti[:])
    nc.compile()
    return nc


def kernel(input, coords):
    input = np.asarray(input, dtype=np.float32)
    coords = np.asarray(coords, dtype=np.float32)
    N = coords.shape[0]

    # ---- int8 quantization (symmetric) + channel-last layout ----
    # round-half-up via +128.5/truncate; uint8^0x80 == int8 value-128
    amax = float(np.abs(input).max())
    if amax == 0.0:
        amax = 1.0
    s = np.float32(amax / 127.0)
    qi = (input * np.float32(127.0 / amax) + np.float32(128.5)) \
        .astype(np.uint8)
    np.bitwise_xor(qi, 128, out=qi)
    qi = qi.view(np.int8)
    Qv = np.ascontiguousarray(qi.reshape(C, -1).T).reshape(D, PLANE, C)

    # ---- binning by x-window (exact same f32 math as the device) ----
    ccdg = (coords + np.float32(1.0)) * np.float32(63.5)
    cx = ccdg[:, 0]
    fx = np.floor(cx).astype(np.int64)
    np.clip(fx, 0, D - 2, out=fx)
    key = fx >> 1                        # global window 0..63
    order = np.argsort(key, kind="stable")
    counts = np.bincount(key, minlength=64)
    capb = max(CH, int(np.ceil(counts.max() / CH)) * CH)
    cpb = capb // CH
    nch = 8 * cpb
    U = nch * (CH // P)

    i_all = np.full(64 * capb, -1, np.int64)  # padded slot -> orig idx
    starts = np.zeros(65, np.int64)
    np.cumsum(counts, out=starts[1:])
    for gb in range(64):
        n = int(counts[gb])
        i_all[gb * capb:gb * capb + n] = order[starts[gb]:starts[gb] + n]

    in_maps = []
    core_meta = []
    binidx = np.arange(8 * capb) // capb
    for c in range(NCORES):
        ids = i_all[c * 8 * capb:(c + 1) * 8 * capb]
        valid = ids >= 0
        # pad coords: center of the bin's first plane, y=z=center;
        # the pad x goes through the same f32 round-trip as a real coord
        padx = ((2 * (8 * c + binidx) + 0.5) / np.float32(63.5)
                - 1.0).astype(np.float32)
        ccd = np.empty((ids.size, 3), np.float32)
        ccd[:, 0] = (padx + np.float32(1.0)) * np.float32(63.5)
        ccd[:, 1:] = np.float32(63.5)
        ccd[valid] = ccdg[ids[valid]]
        xb = (2.0 * (8 * c + binidx)).astype(np.float32)
        q = np.empty((ids.size, 3), np.int16)
        q[:, 0] = np.clip(np.rint((ccd[:, 0] - xb) * np.float32(16384.0)),
                          0, 32767).astype(np.int16)
        q[:, 1] = (np.clip(np.rint(ccd[:, 1] * np.float32(512.0)),
                           0, 65023) - 32768).astype(np.int16)
        q[:, 2] = (np.clip(np.rint(ccd[:, 2] * np.float32(512.0)),
                           0, 65023) - 32768).astype(np.int16)
        # planeA: slot i = k*CH + a*128 + p -> (p, k*64+a)
        arr = q.reshape(nch, CH // P, P, 3).transpose(3, 2, 0, 1)
        pco = np.empty((P, 3 * U), np.int16)
        for d in range(3):
            pco[:, d * U:(d + 1) * U] = arr[d].reshape(P, U)
        if c < NCORES - 1:
            slab = Qv[XPL * c:XPL * c + 17]     # zero-copy view
        else:
            slab = Qv[np.clip(np.arange(XPL * c, XPL * c + 17), 0, D - 1)]
        in_maps.append({
            "slab": slab.reshape(17 * PLANE, C),
            "pco": pco,
        })
        core_meta.append((ids, valid))

    key_cfg = (nch, cpb)
    if key_cfg not in _cache:
        _cache.clear()
        _cache[key_cfg] = _build(nch, cpb)
    nc = _cache[key_cfg]

    import time as _time
    _t0 = _time.perf_counter()
    res = bass_utils.run_bass_kernel_spmd(
        nc, in_maps[:RUN_CORES], core_ids=list(range(RUN_CORES)))
    global LAST_EXEC_S
    LAST_EXEC_S = _time.perf_counter() - _t0
    if RUN_CORES < NCORES:
        z = np.zeros_like(res.results[0]["out"])
        res.results = list(res.results) + [
            {"out": z} for _ in range(NCORES - RUN_CORES)]

    outf = np.empty((N, C), np.float32)
    for c in range(NCORES):
        ids, valid = core_meta[c]
        vals = res.results[c]["out"].reshape(P, nch, CH // P, C)
        vals = vals.transpose(1, 2, 0, 3).reshape(-1, C)
        outf[ids[valid]] = vals[valid].astype(np.float32) * s
    return outf.T


# revision 17
# speedup vs baseline: 1.1345x; 1.1199x over previous
"""Trilinear interpolation (grid_sample) on 8 TRN2 NeuronCores.

The axon tunnel moves ~40MB/s, so wall time is dominated by bytes shipped.
Strategy (vs. shipping a host-built 1GB expanded f32 table):
- Host: quantize the (16,128,128,128) volume to int8 (sym, scale amax/127),
  transpose to channel-last, slice into 8 x-slabs of 17 planes (4.45MB/core).
- Host: bin the 1M points by x-window (2 planes) -> 64 bins, 8 per core;
  pad each bin to a chunk multiple; ship coords once in planeA layout as
  i16 fixed-point (x window-local *2^14; y/z *2^9 biased by -2^15).
- Device: DMA-expand the int8 slab into an 8-corner row table in device DRAM
  (row = 8 corners x 16 ch int8 = 128B payload in a 256B-stride row), with
  y/z edge-clamp handled by split DMAs. DVE computes floor/frac/weights and
  int16 row indices from planeA coords; a DRAM round-trip permutes indices
  into the gpsimd 16-partition-wrapped 8-replica layout; one 256B dma_gather
  per point; DVE converts payload i8->f32, applies corner weights,
  tree-reduces, clamps to [-127,127] and emits int8 output (the HW
  f32->i8 convert rounds to nearest).
- Host: inverse-permute, dequantize by the volume scale.
"""
import numpy as np

try:
    # persistent XLA compile cache: skips re-lowering the PJRT executable on
    # every run_bass_kernel_spmd call (~0.1s/call) and warms fresh processes
    import jax
    jax.config.update("jax_compilation_cache_dir", "/tmp/.jax_neff_cache")
    jax.config.update("jax_persistent_cache_min_entry_size_bytes", 0)
    jax.config.update("jax_persistent_cache_min_compile_time_secs", 0)
except Exception:
    pass

import concourse.bass as bass
import concourse.tile as tile
from concourse import bacc, mybir
from concourse import bass_utils

P = 128
C = 16              # channels
D = 128             # grid size per dim
CH = 8192           # points per gather chunk
ROWB = 256          # bytes per table row (128B payload + 128B pad)
PAY = 128           # payload bytes per row (8 corners * 16 ch int8)
WINDOW = 2 * D * D  # rows per gather window (2 x-planes) = 32768
NCORES = 8
XPL = D // NCORES   # x-planes per core = 16
PLANE = D * D       # 16384 rows per x-plane

_cache = {}
RUN_CORES = 8   # override <8 for debugging: only first k cores run on HW
LAST_EXEC_S = 0.0


def _build(nch, cpb):
    """Build the SPMD Bass program. nch = chunks per core, cpb = chunks per
    bin."""
    U = nch * (CH // P)        # planeA cols per partition (nch*64)
    S = CH // P                # slots per partition per chunk = 64
    TBL = nch * (CH // 16)     # idx table cols (i16), nch*512
    f32 = mybir.dt.float32
    i32 = mybir.dt.int32
    i16 = mybir.dt.int16
    i8 = mybir.dt.int8
    gt = mybir.AluOpType.is_gt

    nc = bacc.Bacc("TRN2", target_bir_lowering=False, debug=False,
                   num_devices=RUN_CORES)
    slab = nc.dram_tensor("slab", [17 * PLANE, C], i8, kind="ExternalInput")
    # fixed-point coords, one i16 plane per dim: x window-local *16384,
    # y/z *512 biased by -32768
    pco = nc.dram_tensor("pco", [P, 3 * U], i16, kind="ExternalInput")
    out = nc.dram_tensor("out", [P, U * C], i8, kind="ExternalOutput")

    def view(ap, dims):
        return bass.AP(ap.tensor, ap.offset, [ap.ap[0]] + dims)

    with tile.TileContext(nc) as tc:
        with tc.tile_pool(name="persist", bufs=1) as pp, \
             tc.tile_pool(name="dram", bufs=1, space="DRAM") as dp:
            # 67MB corner-expanded row table in device DRAM
            table = dp.tile([P, XPL * PLANE * ROWB // P], i8)
            tidx = pp.tile([P, TBL], i16)
            w8 = pp.tile([P, U * 8], f32)
            tv = table[:]
            sv = slab.ap()

            # ---------- corner expansion: slab -> table (DRAM->DRAM) ------
            # table[(x,y,z)*256 + j*16 + ch] = slab[(x+dx, min(y+dy,127),
            #   min(z+dz,127)), ch] for corner j = dx*4+dy*2+dz
            # (DMA APs max 3 dims -> one DMA per x-plane per piece)
            for dx in range(2):
                for dy in range(2):
                    for dz in range(2):
                        j = dx * 4 + dy * 2 + dz
                        ypieces = ([(0, D - 1, 1), (D - 1, D, 0)] if dy
                                   else [(0, D, 0)])
                        zpieces = ([(0, D - 1, 1), (D - 1, D, 0)] if dz
                                   else [(0, D, 0)])
                        for (ys, ye, yd) in ypieces:
                            for (zs, ze, zd) in zpieces:
                                for xl in range(XPL):
                                    woff = (xl * PLANE + ys * D + zs) \
                                        * ROWB + j * C
                                    wdims = [[D * ROWB, ye - ys],
                                             [ROWB, ze - zs], [1, C]]
                                    roff = ((xl + dx) * PLANE
                                            + (ys + yd) * D + (zs + zd)) * C
                                    rdims = [[D * C, ye - ys],
                                             [C, ze - zs], [1, C]]
                                    nc.sync.dma_start(
                                        bass.AP(tv.tensor, tv.offset + woff,
                                                wdims),
                                        bass.AP(sv.tensor, sv.offset + roff,
                                                rdims))

            # ---------- prep: floors/fracs/weights/indices (planeA) -------
            with tc.tile_pool(name="prep", bufs=1) as pa:
                pc = pa.tile([P, 3 * U], i16)
                nc.sync.dma_start(pc[:], pco.ap())

                def floor_frac(d, bias, scale, name):
                    cc = pa.tile([P, U], f32, tag=f"c{name}")
                    nc.vector.tensor_copy(cc[:], pc[:, d * U:(d + 1) * U])
                    nc.vector.tensor_scalar(cc[:], cc[:], bias, scale,
                                            mybir.AluOpType.add,
                                            mybir.AluOpType.mult)
                    fi = pa.tile([P, U], i32, tag=f"fi{name}")
                    nc.vector.tensor_copy(fi[:], cc[:])
                    ff = pa.tile([P, U], f32, tag=f"ff{name}")
                    nc.vector.tensor_copy(ff[:], fi[:])
                    adj = pa.tile([P, U], f32, tag=f"adj{name}")
                    nc.vector.tensor_tensor(adj[:], ff[:], cc[:], gt)
                    nc.vector.tensor_sub(ff[:], ff[:], adj[:])
                    nc.vector.tensor_sub(cc[:], cc[:], ff[:])   # cc = frac
                    return cc, ff

                frx, ffx = floor_frac(0, 0.0, 1.0 / 16384.0, "x")
                fry, ffy = floor_frac(1, 32768.0, 1.0 / 512.0, "y")
                frz, ffz = floor_frac(2, 32768.0, 1.0 / 512.0, "z")

                # idx = px*16384 + ffy*128 + ffz  (fits int16: <= 32767;
                # px = ffx is already the window-local x in {0,1})
                nc.vector.tensor_scalar_mul(ffx[:], ffx[:], float(PLANE))
                nc.vector.tensor_scalar(ffy[:], ffy[:], float(D), None,
                                        mybir.AluOpType.mult)
                nc.vector.tensor_add(ffx[:], ffx[:], ffy[:])
                nc.vector.tensor_add(ffx[:], ffx[:], ffz[:])
                idxi = pa.tile([P, U], i32)
                nc.vector.tensor_copy(idxi[:], ffx[:])
                idx16 = pa.tile([P, U], i16)
                nc.vector.tensor_copy(idx16[:], idxi[:])

                # DRAM round-trip permute into the gpsimd idx-table layout:
                # elem m of chunk k lives at (part 16g + m%16, col k*512 +
                # m//16) for all 8 replicas g. planeA slot (p, k*64+a) is
                # elem m = a*128+p -> L[(p%16)*TBL + k*512 + 8a + p//16]
                L = dp.tile([16, TBL], i16)
                lv = L[:]
                for b in range(8):
                    nc.sync.dma_start(
                        bass.AP(lv.tensor, lv.offset + b,
                                [[TBL, 16], [CH // 16, nch], [8, S]]),
                        idx16[:][16 * b:16 * (b + 1), :])
                for g in range(8):
                    dst = tidx[:][16 * g:16 * (g + 1), :]
                    nc.sync.dma_start(
                        dst, bass.AP(lv.tensor, lv.offset,
                                     [[TBL, 16], [1, TBL]]))

                # corner weights w8[j] = wx[dx]*wy[dy]*wz[dz], j=dx*4+dy*2+dz
                def wpair(fr, name):
                    w = pa.tile([P, U * 2], f32, tag=f"w{name}")
                    wv = w[:].rearrange("p (u two) -> p u two", two=2)
                    nc.vector.tensor_scalar(wv[:, :, 0], fr[:], -1.0, 1.0,
                                            mybir.AluOpType.mult,
                                            mybir.AluOpType.add)
                    nc.vector.tensor_copy(wv[:, :, 1], fr[:])
                    return w

                WX, WY, WZ = wpair(frx, "x"), wpair(fry, "y"), wpair(frz, "z")
                wyz = pa.tile([P, U * 4], f32)
                ay = WY[:]; az = WZ[:]
                nc.vector.tensor_mul(
                    bass.AP(wyz[:].tensor, wyz[:].offset,
                            [wyz[:].ap[0], [4, U], [2, 2], [1, 2]]),
                    bass.AP(ay.tensor, ay.offset,
                            [ay.ap[0], [2, U], [1, 2], [0, 2]]),
                    bass.AP(az.tensor, az.offset,
                            [az.ap[0], [2, U], [0, 2], [1, 2]]))
                ax = WX[:]; ayz = wyz[:]
                nc.vector.tensor_mul(
                    bass.AP(w8[:].tensor, w8[:].offset,
                            [w8[:].ap[0], [8, U], [4, 2], [1, 4]]),
                    bass.AP(ax.tensor, ax.offset,
                            [ax.ap[0], [2, U], [1, 2], [0, 4]]),
                    bass.AP(ayz.tensor, ayz.offset,
                            [ayz.ap[0], [4, U], [0, 2], [1, 4]]))

            # ---------- main loop: gather + weight + reduce + emit --------
            with tc.tile_pool(name="g", bufs=2) as gp, \
                 tc.tile_pool(name="f", bufs=1) as fp, \
                 tc.tile_pool(name="red", bufs=1) as rp, \
                 tc.tile_pool(name="o", bufs=2) as op_:
                for k in range(nch):
                    w = k // cpb
                    g = gp.tile([P, S * ROWB], i8, tag="g")
                    g3 = g[:].rearrange("p (s e) -> p s e", e=ROWB)
                    win = bass.AP(tv.tensor, tv.offset + w * WINDOW * ROWB,
                                  [[ROWB, WINDOW], [1, ROWB]])
                    nc.gpsimd.dma_gather(
                        out_ap=g3, in_ap=win,
                        idxs_ap=tidx[:, k * (CH // 16):(k + 1) * (CH // 16)],
                        num_idxs=CH, num_idxs_reg=CH, elem_size=ROWB,
                        single_packet=False)
                    gf = fp.tile([P, S * PAY], f32, tag="gf")
                    nc.vector.tensor_copy(
                        view(gf[:], [[PAY, S], [1, PAY]]),
                        view(g[:], [[ROWB, S], [1, PAY]]))
                    gv4 = view(gf[:], [[PAY, S], [C, 8], [1, C]])
                    w8v = view(w8[:, k * S * 8:(k + 1) * S * 8],
                               [[8, S], [1, 8], [0, C]])
                    nc.vector.tensor_mul(gv4, gv4, w8v)
                    s1 = rp.tile([P, S * 64], f32, tag="s1")
                    nc.vector.tensor_add(
                        view(s1[:], [[64, S], [1, 64]]),
                        view(gf[:], [[PAY, S], [1, 64]]),
                        view(gf[:, 64:], [[PAY, S], [1, 64]]))
                    s2 = rp.tile([P, S * 32], f32, tag="s2")
                    nc.vector.tensor_add(
                        view(s2[:], [[32, S], [1, 32]]),
                        view(s1[:], [[64, S], [1, 32]]),
                        view(s1[:, 32:], [[64, S], [1, 32]]))
                    ot = rp.tile([P, S * C], f32, tag="ot")
                    nc.vector.tensor_add(
                        view(ot[:], [[C, S], [1, C]]),
                        view(s2[:], [[32, S], [1, C]]),
                        view(s2[:, C:], [[32, S], [1, C]]))
                    # clamp to [-127,127]; the HW f32->i8 convert rounds to
                    # nearest (verified empirically; CoreSim truncates)
                    nc.vector.tensor_scalar_max(ot[:], ot[:], -127.0)
                    nc.vector.tensor_scalar_min(ot[:], ot[:], 127.0)
                    oti = op_.tile([P, S * C], i8, tag="oti")
                    nc.vector.tensor_copy(oti[:], ot[:])
                    nc.sync.dma_start(
                        out.ap()[:, k * S * C:(k + 1) * S * C], oti[:])
    nc.compile()
    return nc


def kernel(input, coords):
    input = np.asarray(input, dtype=np.float32)
    coords = np.asarray(coords, dtype=np.float32)
    N = coords.shape[0]

    # ---- int8 quantization (symmetric) + channel-last layout ----
    # round-half-up via +128.5/truncate; uint8^0x80 == int8 value-128
    amax = float(np.abs(input).max())
    if amax == 0.0:
        amax = 1.0
    s = np.float32(amax / 127.0)
    qi = (input * np.float32(127.0 / amax) + np.float32(128.5)) \
        .astype(np.uint8)
    np.bitwise_xor(qi, 128, out=qi)
    qi = qi.view(np.int8)
    Qv = np.ascontiguousarray(qi.reshape(C, -1).T).reshape(D, PLANE, C)

    # ---- binning by x-window (exact same f32 math as the device) ----
    ccdg = (coords + np.float32(1.0)) * np.float32(63.5)
    cx = ccdg[:, 0]
    fx = np.floor(cx).astype(np.int64)
    np.clip(fx, 0, D - 2, out=fx)
    key = fx >> 1                        # global window 0..63
    order = np.argsort(key, kind="stable")
    counts = np.bincount(key, minlength=64)
    capb = max(CH, int(np.ceil(counts.max() / CH)) * CH)
    cpb = capb // CH
    nch = 8 * cpb
    U = nch * (CH // P)

    i_all = np.full(64 * capb, -1, np.int64)  # padded slot -> orig idx
    starts = np.zeros(65, np.int64)
    np.cumsum(counts, out=starts[1:])
    for gb in range(64):
        n = int(counts[gb])
        i_all[gb * capb:gb * capb + n] = order[starts[gb]:starts[gb] + n]

    in_maps = []
    core_meta = []
    binidx = np.arange(8 * capb) // capb
    for c in range(NCORES):
        ids = i_all[c * 8 * capb:(c + 1) * 8 * capb]
        valid = ids >= 0
        # pad coords: center of the bin's first plane, y=z=center;
        # the pad x goes through the same f32 round-trip as a real coord
        padx = ((2 * (8 * c + binidx) + 0.5) / np.float32(63.5)
                - 1.0).astype(np.float32)
        ccd = np.empty((ids.size, 3), np.float32)
        ccd[:, 0] = (padx + np.float32(1.0)) * np.float32(63.5)
        ccd[:, 1:] = np.float32(63.5)
        ccd[valid] = ccdg[ids[valid]]
        xb = (2.0 * (8 * c + binidx)).astype(np.float32)
        q = np.empty((ids.size, 3), np.int16)
        q[:, 0] = np.clip(np.rint((ccd[:, 0] - xb) * np.float32(16384.0)),
                          0, 32767).astype(np.int16)
        q[:, 1] = (np.clip(np.rint(ccd[:, 1] * np.float32(512.0)),
                           0, 65023) - 32768).astype(np.int16)
        q[:, 2] = (np.clip(np.rint(ccd[:, 2] * np.float32(512.0)),
                           0, 65023) - 32768).astype(np.int16)
        # planeA: slot i = k*CH + a*128 + p -> (p, k*64+a)
        arr = q.reshape(nch, CH // P, P, 3).transpose(3, 2, 0, 1)
        pco = np.empty((P, 3 * U), np.int16)
        for d in range(3):
            pco[:, d * U:(d + 1) * U] = arr[d].reshape(P, U)
        if c < NCORES - 1:
            slab = Qv[XPL * c:XPL * c + 17]     # zero-copy view
        else:
            slab = Qv[np.clip(np.arange(XPL * c, XPL * c + 17), 0, D - 1)]
        in_maps.append({
            "slab": slab.reshape(17 * PLANE, C),
            "pco": pco,
        })
        core_meta.append((ids, valid))

    key_cfg = (nch, cpb)
    if key_cfg not in _cache:
        _cache.clear()
        _cache[key_cfg] = _build(nch, cpb)
    nc = _cache[key_cfg]

    import time as _time
    _t0 = _time.perf_counter()
    res = bass_utils.run_bass_kernel_spmd(
        nc, in_maps[:RUN_CORES], core_ids=list(range(RUN_CORES)))
    global LAST_EXEC_S
    LAST_EXEC_S = _time.perf_counter() - _t0
    if RUN_CORES < NCORES:
        z = np.zeros_like(res.results[0]["out"])
        res.results = list(res.results) + [
            {"out": z} for _ in range(NCORES - RUN_CORES)]

    outf = np.empty((N, C), np.float32)
    for c in range(NCORES):
        ids, valid = core_meta[c]
        vals = res.results[c]["out"].reshape(P, nch, CH // P, C)
        vals = vals.transpose(1, 2, 0, 3).reshape(-1, C)
        outf[ids[valid]] = vals[valid].astype(np.float32) * s
    return outf.T
